# revision 24
# baseline (speedup 1.0000x reference)
"""Causal single-head attention (B=4, S=2048, E=1024, fp32) on 8 TRN2 NeuronCores.

Sharding: data-parallel over batch (4) x 2-way causal-balanced query split.
Core (b, par) handles batch b and query chunks {0,3} (par=0) or {1,2} (par=1)
of 512 rows each.  A per-core host-side permutation of the sequence axis makes
the device program identical on all 8 cores (SPMD):

  par=0: sequence order [c1, c0, c2, c3];  queries at positions [512:1024) and
         [1536:2048) are chunks 0 and 3.
  par=1: sequence order [c0, c1, c3, c2];  queries at the same fixed positions
         are chunks 1 and 2.

block0 attends key positions [0:1024) (8 k-tiles), block1 attends [0:2048)
(16 k-tiles).  Causality = a static triangular mask on the diagonal 512-chunk
(added into the score PSUM via an identity matmul) plus a per-core "dead"
bias (-1e9, folded into the exp activation's per-partition bias) on the key
block the core must not attend (block0/kb0 on par=0, block1/kb2 on par=1).
Weights are replicated; each core computes K/V for its full permuted sequence
and Q only for its 1024 queries.

Device kernel per core (flash-style, no max-subtraction -- scores are
bounded): Q^T/K^T projected with d on partitions, V with k on partitions, all
matmuls in float32r (full PE rate at N>=256; inputs must be produced as f32r
for the walrus verifier); scores S^T per 128-query tile accumulate 8 d-tiles
in PSUM; exp on ScalarE with fused row-sum (accum_out); P transposed 128x128
on the PE; P^T @ V accumulated in PSUM; normalized by the reciprocal row-sum
at the end.  Weight/x DMAs are chunked and dependency-chained so the first
matmul starts ~4us in, and later weight streams don't steal HBM bandwidth
from the critical path.  Measured ~300-310us HW exec per core (max over 8),
absmax-relative error ~3e-4 vs the fp32 reference.
"""

import numpy as np

B, S, E = 4, 2048, 1024
P = 128          # partitions
C = 512          # query chunk
NEG = -1e9
NCORES = 8
SCALE = 1.0 / np.sqrt(np.float32(E))

_CACHE = {}


def _install_drain_patch():
    """walrus in this env fits only 1 sync wait per CTRL_NO instruction; split
    the TileContext end-of-kernel drain waits across trailing SP nops."""
    import concourse.mybir as mybir
    import concourse.tile as tile
    from concourse.vector_clock import ScopedClock

    if getattr(tile.TileContext, "_drain_split_installed", False):
        return

    def _split_drain_and_barrier(self, tick_clock, wait_clock):
        drain_inst = self.nc.sync.drain()
        wait_clock.add_sem_waits(
            drain_inst.ins, ScopedClock({None: tick_clock.global_clock})
        )
        si = drain_inst.ins.sync_info
        waits = list(si.on_wait) if si and si.on_wait else []
        if len(waits) > 1:
            si.on_wait = waits[:1]
            rest = waits[1:]
            while rest:
                chunk, rest = rest[:1], rest[1:]
                nop = self.nc.sync.nop(nofuse=True, hint="drain_wait_split")
                nsi = nop.ins.sync_info
                if nsi is None:
                    nop.ins.sync_info = mybir.SyncInfo(on_wait=chunk, on_update=[])
                else:
                    nsi.on_wait = list(nsi.on_wait) + chunk

        self.nc.all_engine_barrier()
        assert self.sems is not None
        popped = self.nc._tile_sem_poison_stack.pop()
        assert popped is self._sem_poison
        self.nc.clear_and_free_semaphores(list(self.sems.allocated().values()))
        self.nc.all_engine_barrier()

    tile.TileContext._drain_and_barrier = _split_drain_and_barrier
    tile.TileContext._drain_split_installed = True


def _split_excess_waits(nc, limit=1):
    """walrus here fits only `limit` sync waits per instruction; move excess
    waits of every instruction onto injected same-engine NoOps placed directly
    before it (program order on the engine preserves the semantics)."""
    import copy

    import concourse.mybir as mybir

    template = None
    for f in nc.m.functions:
        for bb in f.blocks:
            for inst in bb.instructions:
                if type(inst).__name__ == "InstNoOp":
                    template = inst
                    break
            if template is not None:
                break
        if template is not None:
            break
    assert template is not None, "no InstNoOp template found"

    n = 0
    for f in nc.m.functions:
        for bb in f.blocks:
            new = []
            for inst in bb.instructions:
                si = inst.sync_info
                waits = list(si.on_wait) if si and si.on_wait else []
                if len(waits) > limit:
                    si.on_wait = waits[-limit:]
                    excess = waits[:-limit]
                    while excess:
                        chunk, excess = excess[:limit], excess[limit:]
                        nop = copy.copy(template)
                        nop.name = f"I-wsplit-{n}"
                        n += 1
                        nop.engine = inst.engine
                        nop.sync_info = mybir.SyncInfo(on_wait=chunk, on_update=[])
                        import bass_rust

                        nop.set_nosync_dependencies(
                            bass_rust.InstructionNameOrderedSet()
                        )
                        nop.set_sync_dependencies(
                            bass_rust.InstructionNameOrderedSet()
                        )
                        new.append(nop)
                new.append(inst)
            bb.instructions[:] = new
    return n


def _build_program():
    """One SPMD program; per-core behaviour differs only through input data."""
    import concourse.bass as bass
    import concourse.mybir as mybir
    import concourse.tile as tile
    from concourse.masks import make_identity
    from concourse.tile import add_dep_helper

    _install_drain_patch()

    f32 = mybir.dt.float32
    f32r = mybir.dt.float32r
    Act = mybir.ActivationFunctionType

    nc = bass.Bass(dynamic_dma_scratch_size=128)
    xT = nc.declare_dram_parameter("xT", [E, S], f32r, isOutput=False)
    wq = nc.declare_dram_parameter("wq", [E, E], f32r, isOutput=False)
    wk = nc.declare_dram_parameter("wk", [E, E], f32r, isOutput=False)
    wv = nc.declare_dram_parameter("wv", [E, E], f32r, isOutput=False)
    masks = nc.declare_dram_parameter("masks", [P, 4 * C], f32r, isOutput=False)
    dbias = nc.declare_dram_parameter("dbias", [P, 8], f32, isOutput=False)
    out = nc.declare_dram_parameter("out", [2 * C, E], f32, isOutput=True)

    xT_r = xT.rearrange("(et p) s -> p et s", p=P)      # [128, 8, 2048]
    wq_r = wq.rearrange("(et p) d -> p et d", p=P)      # [128, 8, 1024]
    wk_r = wk.rearrange("(et p) d -> p et d", p=P)
    wv_r = wv.rearrange("(et p) d -> p et d", p=P)

    ET = E // P   # 8 contraction tiles
    DT = E // P   # 8 head-dim tiles
    KTiles = S // P  # 16 key tiles

    with tile.TileContext(nc) as tc:
        from contextlib import ExitStack

        with ExitStack() as ctx:
            big = ctx.enter_context(tc.tile_pool(name="big", bufs=1))
            mpool = ctx.enter_context(tc.tile_pool(name="mask", bufs=1))
            wv0p = ctx.enter_context(tc.tile_pool(name="wv0", bufs=1))
            ident = mpool.tile([P, P], f32)
            make_identity(nc, ident)
            ident_r = mpool.tile([P, P], f32r)
            nc.vector.tensor_copy(ident_r[:], ident[:])
            masks_sb = mpool.tile([P, 4 * C], f32r)
            dbias_sb = mpool.tile([P, 8], f32)
            wv0_sb = wv0p.tile([P, ET, C], f32r, tag="wv0")
            kt_sb = big.tile([P, DT, S], f32r, tag="kt")     # K^T  [d, k]
            qt_sb = big.tile([P, DT, 2 * C], f32r, tag="qt")  # Q^T [d, q]

            # ---- fused K^T + Q^T projection over 512-col chunks of x^T.
            # Weights are DMA'd in per-et chunks so the first matmuls start
            # as soon as the first 512KB lands; Q^T shares the x tiles of
            # chunks 1 and 3 (the fixed query positions).
            with ExitStack() as pctx:
                wkp = pctx.enter_context(tc.tile_pool(name="wk", bufs=1))
                wqp = pctx.enter_context(tc.tile_pool(name="wq", bufs=1))
                xmov = pctx.enter_context(tc.tile_pool(name="xmov", bufs=2))
                ppsum = pctx.enter_context(
                    tc.tile_pool(name="ppsum", bufs=1, space="PSUM")
                )

                wk_sb = wkp.tile([P, ET, E], f32r, tag="wk")
                wq_sb = wqp.tile([P, ET, E], f32r, tag="wq")
                xm0 = xmov.tile([P, ET, C], f32r, tag="xm", name="xm0")
                xm0_d, wk_d, wq_d = [], [], []
                for et in range(ET):
                    xm0_d.append(nc.sync.dma_start(xm0[:, et, :], xT_r[:, et, 0:C]))
                    wk_d.append(nc.sync.dma_start(wk_sb[:, et, :], wk_r[:, et, :]))
                for et in range(ET):
                    wq_d.append(nc.sync.dma_start(wq_sb[:, et, :], wq_r[:, et, :]))
                # wk/xm0 fill in parallel (needed immediately); the later
                # streams (wq, wv0, masks, second x chunk) queue behind them so
                # they don't steal HBM bandwidth from the critical path.
                for i in range(1, len(wq_d)):
                    add_dep_helper(wq_d[i].ins, wq_d[i - 1].ins, reason="dma chain")
                add_dep_helper(wq_d[0].ins, wk_d[-1].ins, reason="dma chain")
                add_dep_helper(wq_d[0].ins, xm0_d[-1].ins, reason="dma chain")
                wv0_d = []
                for et in range(ET):
                    wv0_d.append(
                        nc.sync.dma_start(wv0_sb[:, et, :], wv_r[:, et, 0:C])
                    )
                for i in range(1, ET):
                    add_dep_helper(wv0_d[i].ins, wv0_d[i - 1].ins, reason="dma chain")
                add_dep_helper(wv0_d[0].ins, wq_d[-1].ins, reason="dma chain")
                mask_d = nc.sync.dma_start(masks_sb[:], masks[:])
                nc.sync.dma_start(dbias_sb[:], dbias[:])
                add_dep_helper(mask_d.ins, wv0_d[-1].ins, reason="dma chain")

                for kb in (0, 1, 3, 2):
                    if kb == 0:
                        xm = xm0
                    else:
                        xm = xmov.tile([P, ET, C], f32r, tag="xm")
                        for et in range(ET):
                            d = nc.sync.dma_start(
                                xm[:, et, :], xT_r[:, et, bass.ts(kb, C)]
                            )
                            if kb == 1:
                                add_dep_helper(
                                    d.ins, xm0_d[-1].ins, reason="dma chain"
                                )
                    if True:              # K^T for all keys
                        pps = [
                            ppsum.tile([P, C], f32, tag=f"pp{dt}", name=f"pp{dt}")
                            for dt in range(DT)
                        ]
                        for et in range(ET):
                            for dt in range(DT):
                                nc.tensor.matmul(
                                    pps[dt][:],
                                    wk_sb[:, et, bass.ts(dt, P)],
                                    xm[:, et, :],
                                    start=(et == 0),
                                    stop=(et == ET - 1),
                                )
                        for dt in range(DT):
                            nc.vector.tensor_copy(
                                kt_sb[:, dt, bass.ts(kb, C)], pps[dt][:]
                            )
                    if kb in (1, 3):
                        qb = 0 if kb == 1 else 1
                        qps = [
                            ppsum.tile([P, C], f32, tag=f"pp{dt}", name=f"qp{dt}")
                            for dt in range(DT)
                        ]
                        for et in range(ET):
                            for dt in range(DT):
                                nc.tensor.matmul(
                                    qps[dt][:],
                                    wq_sb[:, et, bass.ts(dt, P)],
                                    xm[:, et, :],
                                    start=(et == 0),
                                    stop=(et == ET - 1),
                                )
                        for dt in range(DT):
                            nc.vector.tensor_copy(
                                qt_sb[:, dt, bass.ts(qb, C)], qps[dt][:]
                            )

            # ---- projection: V (xT stationary, wv moving as 2 512-halves) ----
            bigv = ctx.enter_context(tc.tile_pool(name="bigv", bufs=1))
            v_sb = bigv.tile([P, KTiles, E], f32r, tag="v")   # V   [k, d]
            with ExitStack() as pctx:
                wvp = pctx.enter_context(tc.tile_pool(name="wv", bufs=1))
                xstat = pctx.enter_context(tc.tile_pool(name="xstat", bufs=4))
                vpsum = pctx.enter_context(
                    tc.tile_pool(name="vpsum", bufs=8, space="PSUM")
                )
                wv1_sb = wvp.tile([P, ET, C], f32r, tag="wv1")
                for et in range(ET):
                    nc.sync.dma_start(wv1_sb[:, et, :], wv_r[:, et, C:E])
                wv_halves = [wv0_sb, wv1_sb]
                # all db=0 groups first: the first V matmuls depend only on
                # the preloaded wv0, so the PE never head-of-line blocks on
                # the wv1 DMA that starts at this phase boundary.
                for db in range(2):
                    for kt in range(KTiles):
                        xs = xstat.tile([P, ET, P], f32r, tag="xs")
                        nc.sync.dma_start(xs[:], xT_r[:, :, bass.ts(kt, P)])
                        pp = vpsum.tile([P, C], f32, tag="vpp")
                        for et in range(ET):
                            nc.tensor.matmul(
                                pp[:],
                                xs[:, et, :],
                                wv_halves[db][:, et, :],
                                start=(et == 0),
                                stop=(et == ET - 1),
                            )
                        nc.vector.tensor_copy(v_sb[:, kt, bass.ts(db, C)], pp[:])

            # ---- attention ----
            with ExitStack() as actx:
                ppool = actx.enter_context(tc.tile_pool(name="p", bufs=4))
                ptpool = actx.enter_context(tc.tile_pool(name="pt", bufs=6))
                obuf = actx.enter_context(tc.tile_pool(name="ob", bufs=3))
                stat = actx.enter_context(tc.tile_pool(name="stat", bufs=8))
                spsum = actx.enter_context(
                    tc.tile_pool(name="spsum", bufs=2, space="PSUM")
                )
                opsum = actx.enter_context(
                    tc.tile_pool(name="opsum", bufs=2, space="PSUM")
                )
                ptpsum = actx.enter_context(
                    tc.tile_pool(name="ptpsum", bufs=2, space="PSUM")
                )

                for blk, kext, kborder in ((0, 2, (0, 1)), (1, 4, (0, 1, 2, 3))):
                    for r in range(4):
                        qcols = bass.ds(blk * C + r * P, P)
                        o_lo = opsum.tile([P, C], f32, tag="olo")
                        o_hi = opsum.tile([P, C], f32, tag="ohi")
                        sums = stat.tile([P, 4], f32, tag="sums")
                        for kbi, kb in enumerate(kborder):
                            s_t = spsum.tile([P, C], f32, tag="s")
                            # dead-key mask slot (per-core data), diag slot
                            mask_slots = [r] if kb == kext - 1 else []
                            for dt in range(DT):
                                nc.tensor.matmul(
                                    s_t[:],
                                    qt_sb[:, dt, qcols],
                                    kt_sb[:, dt, bass.ts(kb, C)],
                                    start=(dt == 0),
                                    stop=(dt == DT - 1 and not mask_slots),
                                )
                            for i, slot in enumerate(mask_slots):
                                nc.tensor.matmul(
                                    s_t[:],
                                    ident_r[:],
                                    masks_sb[:, bass.ts(slot, C)],
                                    start=False,
                                    stop=(i == len(mask_slots) - 1),
                                )
                            p_t = ppool.tile([P, C], f32r, tag="p")
                            slot = blk * 4 + kb
                            nc.scalar.activation(
                                p_t[:],
                                s_t[:],
                                Act.Exp,
                                bias=dbias_sb[:, slot : slot + 1],
                                scale=float(SCALE),
                                accum_out=sums[:, kb : kb + 1],
                            )
                            for c4 in range(C // P):
                                kt_idx = kb * (C // P) + c4
                                pt_ps = ptpsum.tile([P, P], f32r, tag="ptps")
                                nc.tensor.transpose(
                                    pt_ps[:], p_t[:, bass.ts(c4, P)], ident_r[:]
                                )
                                pt_sb = ptpool.tile([P, P], f32r, tag="ptsb")
                                nc.vector.tensor_copy(pt_sb[:], pt_ps[:])
                                first = kbi == 0 and c4 == 0
                                last = kbi == kext - 1 and c4 == C // P - 1
                                nc.tensor.matmul(
                                    o_lo[:],
                                    pt_sb[:],
                                    v_sb[:, kt_idx, 0:C],
                                    start=first,
                                    stop=last,
                                )
                                nc.tensor.matmul(
                                    o_hi[:],
                                    pt_sb[:],
                                    v_sb[:, kt_idx, C:E],
                                    start=first,
                                    stop=last,
                                )
                        stot = stat.tile([P, 1], f32, tag="stot")
                        nc.vector.reduce_sum(
                            stot[:], sums[:, 0:kext], axis=mybir.AxisListType.X
                        )
                        recip = stat.tile([P, 1], f32, tag="recip")
                        nc.vector.reciprocal(recip[:], stot[:])
                        ob = obuf.tile([P, E], f32, tag="ob")
                        nc.scalar.activation(
                            ob[:, 0:C], o_lo[:], Act.Copy, scale=recip[:]
                        )
                        nc.scalar.activation(
                            ob[:, C:E], o_hi[:], Act.Copy, scale=recip[:]
                        )
                        nc.sync.dma_start(
                            out[bass.ds((blk * 4 + r) * P, P), :], ob[:]
                        )
    _split_excess_waits(nc)
    return nc


def _chunk_order(par):
    return [1, 0, 2, 3] if par == 0 else [0, 1, 3, 2]


def _build_masks(par):
    m = np.zeros((P, 4, C), np.float32)
    p = np.arange(P)[:, None]
    k = np.arange(C)[None, :]
    for r in range(4):
        m[:, r, :] = np.where(k > P * r + p, np.float32(NEG), np.float32(0.0))
    return np.ascontiguousarray(m.reshape(P, 4 * C))


def _build_dbias(par):
    """Additive exp-bias per (block, kblock) slot: -1e9 kills dead key blocks."""
    d = np.zeros((P, 8), np.float32)
    if par == 0:
        d[:, 0] = NEG      # block0 kb0 dead on par=0
    else:
        d[:, 6] = NEG      # block1 kb2 dead on par=1
    return np.ascontiguousarray(d)


def kernel(x, W_Q, W_K, W_V):
    from concourse.bass_utils import run_bass_kernel_spmd

    x = np.ascontiguousarray(np.asarray(x, dtype=np.float32))
    wqT = np.ascontiguousarray(np.asarray(W_Q, np.float32).T)
    wkT = np.ascontiguousarray(np.asarray(W_K, np.float32).T)
    wvT = np.ascontiguousarray(np.asarray(W_V, np.float32).T)

    if "nc" not in _CACHE:
        _CACHE["nc"] = _build_program()
    nc = _CACHE["nc"]

    in_maps = []
    for c in range(NCORES):
        b, par = c // 2, c % 2
        perm = np.concatenate(
            [np.arange(ch * C, (ch + 1) * C) for ch in _chunk_order(par)]
        )
        xTp = np.ascontiguousarray(x[b][perm].T)  # [E, S]
        in_maps.append(
            {
                "xT": xTp,
                "wq": wqT,
                "wk": wkT,
                "wv": wvT,
                "masks": _build_masks(par),
                "dbias": _build_dbias(par),
            }
        )

    res = run_bass_kernel_spmd(nc, in_maps, list(range(NCORES)))

    out = np.empty((B, S, E), np.float32)
    for c in range(NCORES):
        b, par = c // 2, c % 2
        o = res.results[c]["out"]  # [1024, 1024]
        q0, q1 = ((0, 3) if par == 0 else (1, 2))
        out[b, q0 * C : (q0 + 1) * C] = o[0:C]
        out[b, q1 * C : (q1 + 1) * C] = o[C : 2 * C]
    return out


# revision 25
# speedup vs baseline: 1.0776x; 1.0776x over previous
"""Causal single-head attention (B=4, S=2048, E=1024, fp32) on 8 TRN2 NeuronCores.

Sharding: data-parallel over batch (4) x 2-way causal-balanced query split.
Core (b, par) handles batch b and query chunks {0,3} (par=0) or {1,2} (par=1)
of 512 rows each.  A per-core host-side permutation of the sequence axis makes
the device program identical on all 8 cores (SPMD):

  par=0: sequence order [c1, c0, c2, c3];  queries at positions [512:1024) and
         [1536:2048) are chunks 0 and 3.
  par=1: sequence order [c0, c1, c3, c2];  queries at the same fixed positions
         are chunks 1 and 2.

block0 attends key positions [0:1024) (8 k-tiles), block1 attends [0:2048)
(16 k-tiles).  Causality = a static triangular mask on the diagonal 512-chunk
(added into the score PSUM via an identity matmul) plus a per-core "dead"
bias (-1e9, folded into the exp activation's per-partition bias) on the key
block the core must not attend (block0/kb0 on par=0, block1/kb2 on par=1).
Weights are replicated; each core computes K/V for its full permuted sequence
and Q only for its 1024 queries.

Device kernel per core (flash-style, no max-subtraction -- scores are
bounded): Q^T/K^T projected with d on partitions, V with k on partitions, all
matmuls in float32r (full PE rate at N>=256; inputs must be produced as f32r
for the walrus verifier); scores S^T per 128-query tile accumulate 8 d-tiles
in PSUM; exp on ScalarE with fused row-sum (accum_out); P transposed 128x128
on the PE; P^T @ V accumulated in PSUM; normalized by the reciprocal row-sum
at the end.  Weight/x DMAs are chunked and dependency-chained so the first
matmul starts ~4us in, and later weight streams don't steal HBM bandwidth
from the critical path.  Measured ~300-310us HW exec per core (max over 8),
absmax-relative error ~3e-4 vs the fp32 reference.
"""

import numpy as np

B, S, E = 4, 2048, 1024
P = 128          # partitions
C = 512          # query chunk
NEG = -1e9
NCORES = 8
SCALE = 1.0 / np.sqrt(np.float32(E))

_CACHE = {}


def _install_drain_patch():
    """walrus in this env fits only 1 sync wait per CTRL_NO instruction; split
    the TileContext end-of-kernel drain waits across trailing SP nops."""
    import concourse.mybir as mybir
    import concourse.tile as tile
    from concourse.vector_clock import ScopedClock

    if getattr(tile.TileContext, "_drain_split_installed", False):
        return

    def _split_drain_and_barrier(self, tick_clock, wait_clock):
        drain_inst = self.nc.sync.drain()
        wait_clock.add_sem_waits(
            drain_inst.ins, ScopedClock({None: tick_clock.global_clock})
        )
        si = drain_inst.ins.sync_info
        waits = list(si.on_wait) if si and si.on_wait else []
        if len(waits) > 1:
            si.on_wait = waits[:1]
            rest = waits[1:]
            while rest:
                chunk, rest = rest[:1], rest[1:]
                nop = self.nc.sync.nop(nofuse=True, hint="drain_wait_split")
                nsi = nop.ins.sync_info
                if nsi is None:
                    nop.ins.sync_info = mybir.SyncInfo(on_wait=chunk, on_update=[])
                else:
                    nsi.on_wait = list(nsi.on_wait) + chunk

        self.nc.all_engine_barrier()
        assert self.sems is not None
        popped = self.nc._tile_sem_poison_stack.pop()
        assert popped is self._sem_poison
        self.nc.clear_and_free_semaphores(list(self.sems.allocated().values()))
        self.nc.all_engine_barrier()

    tile.TileContext._drain_and_barrier = _split_drain_and_barrier
    tile.TileContext._drain_split_installed = True


def _split_excess_waits(nc, limit=1):
    """walrus here fits only `limit` sync waits per instruction; move excess
    waits of every instruction onto injected same-engine NoOps placed directly
    before it (program order on the engine preserves the semantics)."""
    import copy

    import concourse.mybir as mybir

    template = None
    for f in nc.m.functions:
        for bb in f.blocks:
            for inst in bb.instructions:
                if type(inst).__name__ == "InstNoOp":
                    template = inst
                    break
            if template is not None:
                break
        if template is not None:
            break
    assert template is not None, "no InstNoOp template found"

    n = 0
    for f in nc.m.functions:
        for bb in f.blocks:
            new = []
            for inst in bb.instructions:
                si = inst.sync_info
                waits = list(si.on_wait) if si and si.on_wait else []
                if len(waits) > limit:
                    si.on_wait = waits[-limit:]
                    excess = waits[:-limit]
                    while excess:
                        chunk, excess = excess[:limit], excess[limit:]
                        nop = copy.copy(template)
                        nop.name = f"I-wsplit-{n}"
                        n += 1
                        nop.engine = inst.engine
                        nop.sync_info = mybir.SyncInfo(on_wait=chunk, on_update=[])
                        import bass_rust

                        nop.set_nosync_dependencies(
                            bass_rust.InstructionNameOrderedSet()
                        )
                        nop.set_sync_dependencies(
                            bass_rust.InstructionNameOrderedSet()
                        )
                        new.append(nop)
                new.append(inst)
            bb.instructions[:] = new
    return n


def _build_program():
    """One SPMD program; per-core behaviour differs only through input data."""
    import concourse.bass as bass
    import concourse.mybir as mybir
    import concourse.tile as tile
    from concourse.masks import make_identity
    from concourse.tile import add_dep_helper

    _install_drain_patch()

    f32 = mybir.dt.float32
    f32r = mybir.dt.float32r
    Act = mybir.ActivationFunctionType

    nc = bass.Bass(dynamic_dma_scratch_size=128)
    xT = nc.declare_dram_parameter("xT", [E, S], f32r, isOutput=False)
    wq = nc.declare_dram_parameter("wq", [E, E], f32r, isOutput=False)
    wk = nc.declare_dram_parameter("wk", [E, E], f32r, isOutput=False)
    wv = nc.declare_dram_parameter("wv", [E, E], f32r, isOutput=False)
    masks = nc.declare_dram_parameter("masks", [P, 4 * C], f32r, isOutput=False)
    dbias = nc.declare_dram_parameter("dbias", [P, 8], f32, isOutput=False)
    out = nc.declare_dram_parameter("out", [2 * C, E], f32, isOutput=True)

    xT_r = xT.rearrange("(et p) s -> p et s", p=P)      # [128, 8, 2048]
    wq_r = wq.rearrange("(et p) d -> p et d", p=P)      # [128, 8, 1024]
    wk_r = wk.rearrange("(et p) d -> p et d", p=P)
    wv_r = wv.rearrange("(et p) d -> p et d", p=P)

    ET = E // P   # 8 contraction tiles
    DT = E // P   # 8 head-dim tiles
    KTiles = S // P  # 16 key tiles

    with tile.TileContext(nc) as tc:
        from contextlib import ExitStack

        with ExitStack() as ctx:
            big = ctx.enter_context(tc.tile_pool(name="big", bufs=1))
            mpool = ctx.enter_context(tc.tile_pool(name="mask", bufs=1))
            wv0p = ctx.enter_context(tc.tile_pool(name="wv0", bufs=1))
            ident = mpool.tile([P, P], f32)
            make_identity(nc, ident)
            ident_r = mpool.tile([P, P], f32r)
            nc.vector.tensor_copy(ident_r[:], ident[:])
            masks_sb = mpool.tile([P, 4 * C], f32r)
            dbias_sb = mpool.tile([P, 8], f32)
            wv0_sb = wv0p.tile([P, ET, C], f32r, tag="wv0")
            kt_sb = big.tile([P, DT, S], f32r, tag="kt")     # K^T  [d, k]
            qt_sb = big.tile([P, DT, 2 * C], f32r, tag="qt")  # Q^T [d, q]

            # ---- fused K^T + Q^T projection over 512-col chunks of x^T.
            # Weights are DMA'd in per-et chunks so the first matmuls start
            # as soon as the first 512KB lands; Q^T shares the x tiles of
            # chunks 1 and 3 (the fixed query positions).
            with ExitStack() as pctx:
                wkp = pctx.enter_context(tc.tile_pool(name="wk", bufs=1))
                wqp = pctx.enter_context(tc.tile_pool(name="wq", bufs=1))
                xmov = pctx.enter_context(tc.tile_pool(name="xmov", bufs=2))
                ppsum = pctx.enter_context(
                    tc.tile_pool(name="ppsum", bufs=1, space="PSUM")
                )

                wk_sb = wkp.tile([P, ET, E], f32r, tag="wk")
                wq_sb = wqp.tile([P, ET, E], f32r, tag="wq")
                xm0 = xmov.tile([P, ET, C], f32r, tag="xm", name="xm0")
                xm0_d, wk_d, wq_d = [], [], []
                for et in range(ET):
                    xm0_d.append(nc.sync.dma_start(xm0[:, et, :], xT_r[:, et, 0:C]))
                    wk_d.append(nc.sync.dma_start(wk_sb[:, et, :], wk_r[:, et, :]))
                for et in range(ET):
                    wq_d.append(nc.sync.dma_start(wq_sb[:, et, :], wq_r[:, et, :]))
                # wk/xm0 fill in parallel (needed immediately); the later
                # streams (wq, wv0, masks, second x chunk) queue behind them so
                # they don't steal HBM bandwidth from the critical path.
                for i in range(1, len(wq_d)):
                    add_dep_helper(wq_d[i].ins, wq_d[i - 1].ins, reason="dma chain")
                add_dep_helper(wq_d[0].ins, wk_d[-1].ins, reason="dma chain")
                add_dep_helper(wq_d[0].ins, xm0_d[-1].ins, reason="dma chain")
                wv0_d = []
                for et in range(ET):
                    wv0_d.append(
                        nc.sync.dma_start(wv0_sb[:, et, :], wv_r[:, et, 0:C])
                    )
                for i in range(1, ET):
                    add_dep_helper(wv0_d[i].ins, wv0_d[i - 1].ins, reason="dma chain")
                add_dep_helper(wv0_d[0].ins, wq_d[-1].ins, reason="dma chain")
                mask_d = nc.sync.dma_start(masks_sb[:], masks[:])
                nc.sync.dma_start(dbias_sb[:], dbias[:])
                add_dep_helper(mask_d.ins, wv0_d[-1].ins, reason="dma chain")

                for kb in (0, 1, 3, 2):
                    if kb == 0:
                        xm = xm0
                    else:
                        xm = xmov.tile([P, ET, C], f32r, tag="xm")
                        for et in range(ET):
                            d = nc.sync.dma_start(
                                xm[:, et, :], xT_r[:, et, bass.ts(kb, C)]
                            )
                            if kb == 1:
                                add_dep_helper(
                                    d.ins, xm0_d[-1].ins, reason="dma chain"
                                )
                    if True:              # K^T for all keys
                        pps = [
                            ppsum.tile([P, C], f32, tag=f"pp{dt}", name=f"pp{dt}")
                            for dt in range(DT)
                        ]
                        for et in range(ET):
                            for dt in range(DT):
                                nc.tensor.matmul(
                                    pps[dt][:],
                                    wk_sb[:, et, bass.ts(dt, P)],
                                    xm[:, et, :],
                                    start=(et == 0),
                                    stop=(et == ET - 1),
                                )
                        for dt in range(DT):
                            nc.vector.tensor_copy(
                                kt_sb[:, dt, bass.ts(kb, C)], pps[dt][:]
                            )
                    if kb in (1, 3):
                        qb = 0 if kb == 1 else 1
                        qps = [
                            ppsum.tile([P, C], f32, tag=f"pp{dt}", name=f"qp{dt}")
                            for dt in range(DT)
                        ]
                        for et in range(ET):
                            for dt in range(DT):
                                nc.tensor.matmul(
                                    qps[dt][:],
                                    wq_sb[:, et, bass.ts(dt, P)],
                                    xm[:, et, :],
                                    start=(et == 0),
                                    stop=(et == ET - 1),
                                )
                        for dt in range(DT):
                            nc.vector.tensor_copy(
                                qt_sb[:, dt, bass.ts(qb, C)], qps[dt][:]
                            )

            # ---- projection: V (xT stationary, wv moving as 2 512-halves) ----
            bigv = ctx.enter_context(tc.tile_pool(name="bigv", bufs=1))
            v_sb = bigv.tile([P, KTiles, E], f32r, tag="v")   # V   [k, d]
            with ExitStack() as pctx:
                wvp = pctx.enter_context(tc.tile_pool(name="wv", bufs=1))
                xstat = pctx.enter_context(tc.tile_pool(name="xstat", bufs=4))
                vpsum = pctx.enter_context(
                    tc.tile_pool(name="vpsum", bufs=8, space="PSUM")
                )
                wv1_sb = wvp.tile([P, ET, C], f32r, tag="wv1")
                for et in range(ET):
                    nc.sync.dma_start(wv1_sb[:, et, :], wv_r[:, et, C:E])
                wv_halves = [wv0_sb, wv1_sb]
                for kt in range(KTiles):
                    xs = xstat.tile([P, ET, P], f32r, tag="xs")
                    nc.sync.dma_start(xs[:], xT_r[:, :, bass.ts(kt, P)])
                    for db in range(2):
                        pp = vpsum.tile([P, C], f32, tag="vpp")
                        for et in range(ET):
                            nc.tensor.matmul(
                                pp[:],
                                xs[:, et, :],
                                wv_halves[db][:, et, :],
                                start=(et == 0),
                                stop=(et == ET - 1),
                            )
                        nc.vector.tensor_copy(v_sb[:, kt, bass.ts(db, C)], pp[:])

            # ---- attention ----
            with ExitStack() as actx:
                ppool = actx.enter_context(tc.tile_pool(name="p", bufs=4))
                ptpool = actx.enter_context(tc.tile_pool(name="pt", bufs=6))
                obuf = actx.enter_context(tc.tile_pool(name="ob", bufs=2))
                stat = actx.enter_context(tc.tile_pool(name="stat", bufs=8))
                spsum = actx.enter_context(
                    tc.tile_pool(name="spsum", bufs=2, space="PSUM")
                )
                opsum = actx.enter_context(
                    tc.tile_pool(name="opsum", bufs=2, space="PSUM")
                )
                ptpsum = actx.enter_context(
                    tc.tile_pool(name="ptpsum", bufs=2, space="PSUM")
                )

                for blk, kext, kborder in ((0, 2, (0, 1)), (1, 4, (0, 1, 2, 3))):
                    for r in range(4):
                        qcols = bass.ds(blk * C + r * P, P)
                        o_lo = opsum.tile([P, C], f32, tag="olo")
                        o_hi = opsum.tile([P, C], f32, tag="ohi")
                        sums = stat.tile([P, 4], f32, tag="sums")
                        for kbi, kb in enumerate(kborder):
                            s_t = spsum.tile([P, C], f32, tag="s")
                            # dead-key mask slot (per-core data), diag slot
                            mask_slots = [r] if kb == kext - 1 else []
                            for dt in range(DT):
                                nc.tensor.matmul(
                                    s_t[:],
                                    qt_sb[:, dt, qcols],
                                    kt_sb[:, dt, bass.ts(kb, C)],
                                    start=(dt == 0),
                                    stop=(dt == DT - 1 and not mask_slots),
                                )
                            for i, slot in enumerate(mask_slots):
                                nc.tensor.matmul(
                                    s_t[:],
                                    ident_r[:],
                                    masks_sb[:, bass.ts(slot, C)],
                                    start=False,
                                    stop=(i == len(mask_slots) - 1),
                                )
                            p_t = ppool.tile([P, C], f32r, tag="p")
                            slot = blk * 4 + kb
                            nc.scalar.activation(
                                p_t[:],
                                s_t[:],
                                Act.Exp,
                                bias=dbias_sb[:, slot : slot + 1],
                                scale=float(SCALE),
                                accum_out=sums[:, kb : kb + 1],
                            )
                            for c4 in range(C // P):
                                kt_idx = kb * (C // P) + c4
                                pt_ps = ptpsum.tile([P, P], f32r, tag="ptps")
                                nc.tensor.transpose(
                                    pt_ps[:], p_t[:, bass.ts(c4, P)], ident_r[:]
                                )
                                pt_sb = ptpool.tile([P, P], f32r, tag="ptsb")
                                nc.vector.tensor_copy(pt_sb[:], pt_ps[:])
                                first = kbi == 0 and c4 == 0
                                last = kbi == kext - 1 and c4 == C // P - 1
                                nc.tensor.matmul(
                                    o_lo[:],
                                    pt_sb[:],
                                    v_sb[:, kt_idx, 0:C],
                                    start=first,
                                    stop=last,
                                )
                                nc.tensor.matmul(
                                    o_hi[:],
                                    pt_sb[:],
                                    v_sb[:, kt_idx, C:E],
                                    start=first,
                                    stop=last,
                                )
                        stot = stat.tile([P, 1], f32, tag="stot")
                        nc.vector.reduce_sum(
                            stot[:], sums[:, 0:kext], axis=mybir.AxisListType.X
                        )
                        recip = stat.tile([P, 1], f32, tag="recip")
                        nc.vector.reciprocal(recip[:], stot[:])
                        ob = obuf.tile([P, E], f32, tag="ob")
                        nc.scalar.activation(
                            ob[:, 0:C], o_lo[:], Act.Copy, scale=recip[:]
                        )
                        nc.scalar.activation(
                            ob[:, C:E], o_hi[:], Act.Copy, scale=recip[:]
                        )
                        nc.sync.dma_start(
                            out[bass.ds((blk * 4 + r) * P, P), :], ob[:]
                        )
    _split_excess_waits(nc)
    return nc


def _chunk_order(par):
    return [1, 0, 2, 3] if par == 0 else [0, 1, 3, 2]


def _build_masks(par):
    m = np.zeros((P, 4, C), np.float32)
    p = np.arange(P)[:, None]
    k = np.arange(C)[None, :]
    for r in range(4):
        m[:, r, :] = np.where(k > P * r + p, np.float32(NEG), np.float32(0.0))
    return np.ascontiguousarray(m.reshape(P, 4 * C))


def _build_dbias(par):
    """Additive exp-bias per (block, kblock) slot: -1e9 kills dead key blocks."""
    d = np.zeros((P, 8), np.float32)
    if par == 0:
        d[:, 0] = NEG      # block0 kb0 dead on par=0
    else:
        d[:, 6] = NEG      # block1 kb2 dead on par=1
    return np.ascontiguousarray(d)


def kernel(x, W_Q, W_K, W_V):
    from concourse.bass_utils import run_bass_kernel_spmd

    x = np.ascontiguousarray(np.asarray(x, dtype=np.float32))
    wqT = np.ascontiguousarray(np.asarray(W_Q, np.float32).T)
    wkT = np.ascontiguousarray(np.asarray(W_K, np.float32).T)
    wvT = np.ascontiguousarray(np.asarray(W_V, np.float32).T)

    if "nc" not in _CACHE:
        _CACHE["nc"] = _build_program()
    nc = _CACHE["nc"]

    in_maps = []
    for c in range(NCORES):
        b, par = c // 2, c % 2
        perm = np.concatenate(
            [np.arange(ch * C, (ch + 1) * C) for ch in _chunk_order(par)]
        )
        xTp = np.ascontiguousarray(x[b][perm].T)  # [E, S]
        in_maps.append(
            {
                "xT": xTp,
                "wq": wqT,
                "wk": wkT,
                "wv": wvT,
                "masks": _build_masks(par),
                "dbias": _build_dbias(par),
            }
        )

    res = run_bass_kernel_spmd(nc, in_maps, list(range(NCORES)))

    out = np.empty((B, S, E), np.float32)
    for c in range(NCORES):
        b, par = c // 2, c % 2
        o = res.results[c]["out"]  # [1024, 1024]
        q0, q1 = ((0, 3) if par == 0 else (1, 2))
        out[b, q0 * C : (q0 + 1) * C] = o[0:C]
        out[b, q1 * C : (q1 + 1) * C] = o[C : 2 * C]
    return out
